# revision 1
# baseline (speedup 1.0000x reference)
"""Causal self-attention (single-head, d_model=512) on 8 Trainium2 cores.

Problem: x[4,4096,512] fp32, w_qkv[1536,512], w_proj[512,512]
  qkv = x @ w_qkv.T; scores = q k^T / sqrt(512) causal-masked; softmax;
  out = (softmax @ v) @ w_proj.T

Sharding: 2 cores per batch. Each core owns 4 query blocks of 512 rows,
chosen so both cores' causal work fits one shared SPMD extent profile
E = [8,16,24,32] key-tiles (of 128):
  part 0: global blocks [0,2,5,7] (demands 4,12,24,32 k-tiles)
  part 1: global blocks [1,3,4,6] (demands 8,16,20,28 k-tiles)
Only k-tiles >= MASK_START[i] = [0,8,16,24] can be non-fully-visible for
either core; those get a data-driven mask built from per-row global
indices (host input) vs per-key indices (host input).

Layouts avoid every on-chip transpose:
  scores^T[key,row] = (kT chunk).T @ (qT chunk)  -- kT/qT are [C,*] layouts
  out^T[c,row]     += (v tile).T @ exp(scores^T) -- v natural [key,C] layout
  y[row,o]          = (out^T chunk).T @ w_proj^T chunk
Row-sums of exp come from a ones-vector matmul accumulated in PSUM.
Softmax skips max-subtraction (scores are ~N(0,1); exp is safe in fp32).
"""

import numpy as np
import ml_dtypes
from contextlib import ExitStack

import concourse.bass as bass
import concourse.mybir as mybir
import concourse.tile as tile

B, T, C = 4, 4096, 512
NCORES = 8
QB = 512  # query block rows
QR = 4 * QB  # rows per core
CC = C // 128  # contraction chunks (4)
TK = T // 128  # key tiles (32)
E_PROF = [8, 16, 24, 32]  # baked k-extent per local slot
MASK_START = [0, 8, 16, 24]  # first k-tile needing a mask, per slot
ASSIGN = {0: [0, 2, 5, 7], 1: [1, 3, 4, 6]}  # part -> global block ids
SCALE = 1.0 / np.sqrt(C)

BF = mybir.dt.bfloat16
F32 = mybir.dt.float32
BFNP = ml_dtypes.bfloat16


def _split_excess_waits(nc, max_waits=1):
    """The walrus build in this env rejects >1 sync-wait command on one
    instruction; hoist extras onto standalone same-engine NoOps."""
    for _, bbb in nc.bb_map.items():
        bb = bbb.bb
        new = []
        for inst in list(bb.instructions):
            si = inst.sync_info
            waits = list(si.on_wait) if si and si.on_wait else []
            if len(waits) > max_waits:
                for j, w in enumerate(waits[max_waits:]):
                    new.append(
                        mybir.InstNoOp(
                            name=f"{inst.name}-hw{j}",
                            engine=inst.engine,
                            sync_info=mybir.SyncInfo(on_wait=[w], on_update=[]),
                        )
                    )
                si.on_wait = waits[:max_waits]
                inst.sync_info = si
            new.append(inst)
        bb.instructions = new


def build_program():
    nc = bass.Bass()
    d_xT = nc.dram_tensor("xT", [C, T], BF, kind="ExternalInput")
    d_qxT = nc.dram_tensor("qxT", [C, QR], BF, kind="ExternalInput")
    d_wq = nc.dram_tensor("wqT", [C, C], BF, kind="ExternalInput")
    d_wk = nc.dram_tensor("wkT", [C, C], BF, kind="ExternalInput")
    d_wv = nc.dram_tensor("wvT", [C, C], BF, kind="ExternalInput")
    d_wp = nc.dram_tensor("wpT", [C, C], BF, kind="ExternalInput")
    d_ri = nc.dram_tensor("ri", [128, QR], F32, kind="ExternalInput")
    d_ki = nc.dram_tensor("ki", [128, TK], F32, kind="ExternalInput")
    d_y = nc.dram_tensor("y", [QR, C], F32, kind="ExternalOutput")

    with tile.TileContext(nc) as tc:
        with ExitStack() as ctx:
            const = ctx.enter_context(tc.tile_pool(name="const", bufs=1))
            work = ctx.enter_context(tc.tile_pool(name="work", bufs=3))

            # ---- persistent SBUF tensors ----
            xts = [
                const.tile([128, CC, 512], BF, tag=f"xt{kb}", name=f"xt{kb}")
                for kb in range(T // 512)
            ]
            qxt = const.tile([128, CC, QR], BF, tag="qxt")
            wq = const.tile([128, CC, C], BF, tag="wq")
            wk = const.tile([128, CC, C], BF, tag="wk")
            wv = const.tile([128, CC, C], BF, tag="wv")
            wp = const.tile([128, CC, C], BF, tag="wp")
            ri = const.tile([128, QR], F32, tag="ri")
            ki = const.tile([128, TK], F32, tag="ki")
            kt = const.tile([128, CC, T], BF, tag="kt")
            vt = const.tile([128, TK, C], BF, tag="vt")
            qt = const.tile([128, CC, QR], BF, tag="qt")
            ones = const.tile([128, 1], BF, tag="ones")
            rr = const.tile([128, 16], F32, tag="rr")  # 1/rowsum, [p, slot*4+rt]

            xT_r = d_xT.ap().rearrange("(c p) t -> p c t", p=128)
            qxT_r = d_qxT.ap().rearrange("(c p) t -> p c t", p=128)
            nc.sync.dma_start(wk[:], d_wk.ap().rearrange("(c p) o -> p c o", p=128))
            nc.sync.dma_start(wv[:], d_wv.ap().rearrange("(c p) o -> p c o", p=128))
            for kb in range(T // 512):
                nc.sync.dma_start(
                    xts[kb][:], xT_r[:, :, kb * 512 : (kb + 1) * 512]
                )
            nc.sync.dma_start(wq[:], d_wq.ap().rearrange("(c p) o -> p c o", p=128))
            for qb in range(QR // 512):
                nc.sync.dma_start(
                    qxt[:, :, qb * 512 : (qb + 1) * 512],
                    qxT_r[:, :, qb * 512 : (qb + 1) * 512],
                )
            nc.sync.dma_start(wp[:], d_wp.ap().rearrange("(c p) o -> p c o", p=128))
            nc.sync.dma_start(ri[:], d_ri.ap())
            nc.sync.dma_start(ki[:], d_ki.ap())
            nc.gpsimd.memset(ones[:], 1.0)

            # ---- phase B: q/k/v production (kb-outer so compute starts
            # after the first x chunk lands, not after the full load) ----
            with tc.tile_pool(name="ps_qkv", bufs=4, space="PSUM") as ps_qkv:
                for kb in range(T // 512):
                    # kT[c_out, key] tiles for this key block
                    for oc in range(CC):
                        ps = ps_qkv.tile([128, 512], F32, tag="qkv")
                        for cc in range(CC):
                            nc.tensor.matmul(
                                ps[:],
                                lhsT=wk[:, cc, oc * 128 : (oc + 1) * 128],
                                rhs=xts[kb][:, cc, :],
                                start=(cc == 0),
                                stop=(cc == CC - 1),
                            )
                        nc.scalar.copy(kt[:, oc, kb * 512 : (kb + 1) * 512], ps[:])
                    # v[key, c] tiles for this key block
                    for kv in range(4 * kb, 4 * kb + 4):
                        ps = ps_qkv.tile([128, 512], F32, tag="qkv")
                        for cc in range(CC):
                            nc.tensor.matmul(
                                ps[:],
                                lhsT=xts[kv // 4][:, cc, (kv % 4) * 128 : (kv % 4 + 1) * 128],
                                rhs=wv[:, cc, :],
                                start=(cc == 0),
                                stop=(cc == CC - 1),
                            )
                        nc.vector.tensor_copy(vt[:, kv, :], ps[:])
                # qT[c_out, row] tiles (wq pre-scaled by 1/sqrt(C) on host)
                for oc in range(CC):
                    for qb in range(QR // 512):
                        ps = ps_qkv.tile([128, 512], F32, tag="qkv")
                        for cc in range(CC):
                            nc.tensor.matmul(
                                ps[:],
                                lhsT=wq[:, cc, oc * 128 : (oc + 1) * 128],
                                rhs=qxt[:, cc, qb * 512 : (qb + 1) * 512],
                                start=(cc == 0),
                                stop=(cc == CC - 1),
                            )
                        nc.scalar.copy(qt[:, oc, qb * 512 : (qb + 1) * 512], ps[:])

            # ---- phases C+D: attention + projection per slot ----
            with tc.tile_pool(name="ps_at", bufs=1, space="PSUM") as ps_at:
                for i in range(4):
                    E = E_PROF[i]
                    ms = MASK_START[i]
                    ot = [
                        ps_at.tile([128, 512], F32, tag=f"ot{cc}", name=f"ot{cc}")
                        for cc in range(CC)
                    ]
                    rs = ps_at.tile([1, 512], F32, tag="rs")

                    def emit_scores(k):
                        st = ps_at.tile([128, 512], F32, tag="st", bufs=2, name="st")
                        for cc in range(CC):
                            nc.tensor.matmul(
                                st[:],
                                lhsT=kt[:, cc, k * 128 : (k + 1) * 128],
                                rhs=qt[:, cc, i * 512 : (i + 1) * 512],
                                start=(cc == 0),
                                stop=(cc == CC - 1),
                            )
                        return st

                    # software pipeline: scores(k+1) issue on PE before the
                    # exp(k)-dependent WV matmuls, hiding the ACT latency
                    st_cur = emit_scores(0)
                    for k in range(E):
                        st_nxt = emit_scores(k + 1) if k + 1 < E else None
                        e = work.tile([128, 512], BF, tag="e")
                        if k < ms:
                            nc.scalar.activation(
                                e[:], st_cur[:], mybir.ActivationFunctionType.Exp
                            )
                        else:
                            ef = work.tile([128, 512], F32, tag="ef")
                            nc.scalar.activation(
                                ef[:], st_cur[:], mybir.ActivationFunctionType.Exp
                            )
                            m = work.tile([128, 512], F32, tag="m")
                            nc.vector.tensor_scalar(
                                m[:],
                                in0=ri[:, i * 512 : (i + 1) * 512],
                                scalar1=ki[:, k : k + 1],
                                scalar2=None,
                                op0=mybir.AluOpType.is_ge,
                            )
                            nc.vector.tensor_tensor(
                                e[:], ef[:], m[:], op=mybir.AluOpType.mult
                            )
                        nc.tensor.matmul(
                            rs[:],
                            lhsT=ones[:],
                            rhs=e[:],
                            start=(k == 0),
                            stop=(k == E - 1),
                        )
                        for cc in range(CC):
                            nc.tensor.matmul(
                                ot[cc][:],
                                lhsT=vt[:, k, cc * 128 : (cc + 1) * 128],
                                rhs=e[:],
                                start=(k == 0),
                                stop=(k == E - 1),
                            )
                        st_cur = st_nxt
                    # evacuate out^T, build 1/rowsum in row-partition layout
                    otsb = work.tile([128, CC, 512], BF, tag="otsb")
                    for cc in range(CC):
                        eng = nc.scalar if cc % 2 == 0 else nc.vector
                        if cc % 2 == 0:
                            nc.scalar.copy(otsb[:, cc, :], ot[cc][:])
                        else:
                            nc.vector.tensor_copy(otsb[:, cc, :], ot[cc][:])
                    rsb = work.tile([1, 512], F32, tag="rsb")
                    nc.vector.tensor_copy(rsb[:], rs[:])
                    rpp = work.tile([128, 4], F32, tag="rpp")
                    for rt in range(4):
                        nc.gpsimd.dma_start(
                            rpp[:, rt : rt + 1], rsb[0:1, rt * 128 : (rt + 1) * 128]
                        )
                    nc.vector.reciprocal(rr[:, i * 4 : (i + 1) * 4], rpp[:])
                    # projection
                    for rt in range(4):
                        yp = ps_at.tile([128, 512], F32, tag="yp")
                        for cc in range(CC):
                            nc.tensor.matmul(
                                yp[:],
                                lhsT=otsb[:, cc, rt * 128 : (rt + 1) * 128],
                                rhs=wp[:, cc, :],
                                start=(cc == 0),
                                stop=(cc == CC - 1),
                            )
                        ysb = work.tile([128, 512], F32, tag="ysb")
                        nc.vector.tensor_scalar(
                            ysb[:],
                            in0=yp[:],
                            scalar1=rr[:, i * 4 + rt : i * 4 + rt + 1],
                            scalar2=None,
                            op0=mybir.AluOpType.mult,
                        )
                        r0 = i * 512 + rt * 128
                        nc.sync.dma_start(d_y.ap()[r0 : r0 + 128, :], ysb[:])

    _split_excess_waits(nc)
    return nc


_NC = None


def _get_program():
    global _NC
    if _NC is None:
        _NC = build_program()
    return _NC


LAST_RESULT = None


def kernel(x, w_qkv, w_proj):
    from concourse.bass_utils import run_bass_kernel_spmd

    x = np.asarray(x, dtype=np.float32)
    w_qkv = np.asarray(w_qkv, dtype=np.float32)
    w_proj = np.asarray(w_proj, dtype=np.float32)

    wqT = np.ascontiguousarray((w_qkv[0:C] * SCALE).T).astype(BFNP)
    wkT = np.ascontiguousarray(w_qkv[C : 2 * C].T).astype(BFNP)
    wvT = np.ascontiguousarray(w_qkv[2 * C : 3 * C].T).astype(BFNP)
    wpT = np.ascontiguousarray(w_proj.T).astype(BFNP)
    ki = np.broadcast_to(
        np.arange(0, T, 128, dtype=np.float32)[None, :], (128, TK)
    ) + np.arange(128, dtype=np.float32)[:, None] * 0
    ki = (np.arange(128, dtype=np.float32)[:, None]
          + np.arange(0, T, 128, dtype=np.float32)[None, :])  # key idx = p + 128*k
    ki = np.ascontiguousarray(ki, dtype=np.float32)

    in_maps = []
    for core in range(NCORES):
        b, part = divmod(core, 2)
        blocks = ASSIGN[part]
        xT = np.ascontiguousarray(x[b].T).astype(BFNP)
        qx = np.concatenate([x[b, qb * QB : (qb + 1) * QB, :] for qb in blocks], 0)
        qxT = np.ascontiguousarray(qx.T).astype(BFNP)
        rvals = np.concatenate(
            [np.arange(qb * QB, (qb + 1) * QB, dtype=np.float32) for qb in blocks]
        )
        ri = np.ascontiguousarray(np.broadcast_to(rvals[None, :], (128, QR)))
        in_maps.append(
            {
                "xT": xT,
                "qxT": qxT,
                "wqT": wqT,
                "wkT": wkT,
                "wvT": wvT,
                "wpT": wpT,
                "ri": ri,
                "ki": ki,
            }
        )

    global LAST_RESULT
    res = run_bass_kernel_spmd(_get_program(), in_maps, core_ids=list(range(NCORES)))
    LAST_RESULT = res

    y = np.empty((B, T, C), dtype=np.float32)
    for core in range(NCORES):
        b, part = divmod(core, 2)
        yc = res.results[core]["y"]
        for i, qb in enumerate(ASSIGN[part]):
            y[b, qb * QB : (qb + 1) * QB, :] = yc[i * QB : (i + 1) * QB, :]
    return y



# revision 11
# speedup vs baseline: 1.1858x; 1.1858x over previous
"""Causal self-attention (single-head, d_model=512) on 8 Trainium2 cores.

Problem: x[4,4096,512] fp32, w_qkv[1536,512], w_proj[512,512]
  qkv = x @ w_qkv.T; scores = q k^T / sqrt(512) causal-masked; softmax;
  out = (softmax @ v) @ w_proj.T

Sharding: 2 cores per batch, 256-row query blocks. Both cores of a pair
run the same slot schedule g=0..7 with key-tile extents E=4g+4; core
part 0 owns rows [512g, 512g+256), part 1 owns [512g+256, 512g+512).
Part-0 extents are 4g+2 (2 tiles of padding, masked off), part-1
extents are 4g+4 exactly. The last tile of each slot streams only 128
rows (causal diagonal trim); tiles 4g..4g+3 are masked via
host-precomputed bf16 masks.

Layouts avoid every on-chip transpose:
  scores^T[key,row] = (kT chunk).T @ (qT chunk)  -- kT/qT are [C,*] layouts
  out^T[c,row]     += (v tile).T @ exp(scores^T) -- v natural [key,C] layout
  y[row,o]          = (out^T chunk).T @ w_proj^T chunk
Row-sums of exp are accumulated on DVE+Pool engines into SBUF, reduced
across partitions once per slot by a tiny fp32 ones-matmul. Softmax
skips max-subtraction (scores are ~N(0,1); exp is safe in fp32).
"""

import numpy as np
import ml_dtypes
from contextlib import ExitStack

import concourse.bass as bass
import concourse.mybir as mybir
import concourse.tile as tile

B, T, C = 4, 4096, 512
NCORES = 8
CC = C // 128  # contraction chunks (4)
TK = T // 128  # key tiles (32)
NSLOT = 8  # 256-row query blocks per core
SCALE = 1.0 / np.sqrt(C)

BF = mybir.dt.bfloat16
F32 = mybir.dt.float32
BFNP = ml_dtypes.bfloat16

# per-slot schedule: E tiles, last one streams 128 rows, rest 256
E_OF = [4 * g + 4 for g in range(NSLOT)]
MS_OF = [4 * g for g in range(NSLOT)]  # first masked tile


def _tile_width(g, k):
    """streamed row-columns for slot g, key-tile k (k < E_OF[g])"""
    return 128 if k == E_OF[g] - 1 else 256


def _mask_cols():
    off = {}
    tot = 0
    for g in range(NSLOT):
        for k in range(MS_OF[g], E_OF[g]):
            off[(g, k)] = tot
            tot += _tile_width(g, k)
    return off, tot


MASK_OFF, MASK_TOT = _mask_cols()  # 8 slots x 4 tiles, widths 256/256/256/128


def _split_excess_waits(nc, max_waits=1):
    """The walrus build in this env rejects >1 sync-wait command on one
    instruction; hoist extras onto standalone same-engine NoOps."""
    for _, bbb in nc.bb_map.items():
        bb = bbb.bb
        new = []
        for inst in list(bb.instructions):
            si = inst.sync_info
            waits = list(si.on_wait) if si and si.on_wait else []
            if len(waits) > max_waits:
                for j, w in enumerate(waits[max_waits:]):
                    new.append(
                        mybir.InstNoOp(
                            name=f"{inst.name}-hw{j}",
                            engine=inst.engine,
                            sync_info=mybir.SyncInfo(on_wait=[w], on_update=[]),
                        )
                    )
                si.on_wait = waits[:max_waits]
                inst.sync_info = si
            new.append(inst)
        bb.instructions = new


def build_program():
    nc = bass.Bass()
    # host-prepped contiguous layouts (partition dim first)
    d_xt = nc.dram_tensor("xt", [128, 8, CC, 512], BF, kind="ExternalInput")
    d_qxt = nc.dram_tensor("qxt", [128, 4, CC, 512], BF, kind="ExternalInput")
    d_wq = nc.dram_tensor("wq", [128, CC, C], BF, kind="ExternalInput")
    d_wk = nc.dram_tensor("wk", [128, CC, C], BF, kind="ExternalInput")
    d_wv = nc.dram_tensor("wv", [128, CC, C], BF, kind="ExternalInput")
    d_wp = nc.dram_tensor("wp", [128, CC, C], BF, kind="ExternalInput")
    d_mask = nc.dram_tensor("mask", [128, MASK_TOT], BF, kind="ExternalInput")
    d_y = nc.dram_tensor("y", [2048, C], F32, kind="ExternalOutput")

    with tile.TileContext(nc) as tc:
        with ExitStack() as ctx:
            const = ctx.enter_context(tc.tile_pool(name="const", bufs=1))
            work = ctx.enter_context(tc.tile_pool(name="work", bufs=3))

            # ---- persistent SBUF tensors ----
            xts = [
                const.tile([128, CC, 512], BF, tag=f"xt{kb}", name=f"xt{kb}")
                for kb in range(8)
            ]
            qxt = const.tile([128, 4, CC, 512], BF, tag="qxt")
            wq = const.tile([128, CC, C], BF, tag="wq")
            wk = const.tile([128, CC, C], BF, tag="wk")
            wv = const.tile([128, CC, C], BF, tag="wv")
            wp = const.tile([128, CC, C], BF, tag="wp")
            maskt = const.tile([128, MASK_TOT], BF, tag="mask")
            kt = const.tile([128, CC, T], BF, tag="kt")
            vt = const.tile([128, TK, C], BF, tag="vt")
            qt = const.tile([128, CC, 2048], BF, tag="qt")
            ones = const.tile([128, 1], F32, tag="ones")
            racc_v = const.tile([128, 256], F32, tag="racc_v")
            racc_p = const.tile([128, 256], F32, tag="racc_p")
            rr = const.tile([128, 16], F32, tag="rr")  # 1/rowsum [p, slot*2+rt]

            # input DMAs, first-needed first
            nc.sync.dma_start(wk[:], d_wk.ap())
            nc.sync.dma_start(xts[0][:], d_xt.ap()[:, 0])
            nc.sync.dma_start(wv[:], d_wv.ap())
            for kb in range(1, 8):
                nc.sync.dma_start(xts[kb][:], d_xt.ap()[:, kb])
            nc.sync.dma_start(wq[:], d_wq.ap())
            nc.sync.dma_start(qxt[:], d_qxt.ap())
            nc.sync.dma_start(wp[:], d_wp.ap())
            nc.sync.dma_start(maskt[:], d_mask.ap())
            nc.gpsimd.memset(ones[:], 1.0)

            # ---- phase B: q/k/v production ----
            with tc.tile_pool(name="ps_qkv", bufs=4, space="PSUM") as ps_qkv:
                for kb in range(8):
                    # kT[oc, key] for this 512-key block
                    for oc in range(CC):
                        ps = ps_qkv.tile([128, 512], F32, tag="qkv")
                        for cc in range(CC):
                            nc.tensor.matmul(
                                ps[:],
                                lhsT=wk[:, cc, oc * 128 : (oc + 1) * 128],
                                rhs=xts[kb][:, cc, :],
                                start=(cc == 0),
                                stop=(cc == CC - 1),
                            )
                        nc.scalar.copy(kt[:, oc, kb * 512 : (kb + 1) * 512], ps[:])
                    # v[key, c] tiles for this block
                    for j in range(4):
                        t = kb * 4 + j
                        ps = ps_qkv.tile([128, 512], F32, tag="qkv")
                        for cc in range(CC):
                            nc.tensor.matmul(
                                ps[:],
                                lhsT=xts[kb][:, cc, j * 128 : (j + 1) * 128],
                                rhs=wv[:, cc, :],
                                start=(cc == 0),
                                stop=(cc == CC - 1),
                            )
                        nc.vector.tensor_copy(vt[:, t, :], ps[:])
                # qT[oc, row] (wq pre-scaled by 1/sqrt(C) on host)
                for oc in range(CC):
                    for qb in range(4):
                        ps = ps_qkv.tile([128, 512], F32, tag="qkv")
                        for cc in range(CC):
                            nc.tensor.matmul(
                                ps[:],
                                lhsT=wq[:, cc, oc * 128 : (oc + 1) * 128],
                                rhs=qxt[:, qb, cc, :],
                                start=(cc == 0),
                                stop=(cc == CC - 1),
                            )
                        nc.scalar.copy(qt[:, oc, qb * 512 : (qb + 1) * 512], ps[:])

            # ---- phases C+D: attention + projection, flat pipeline ----
            # Flat (slot, tile) schedule with 2-tile score lookahead so the
            # exp->PV dependency latency is hidden by the next scores on PE,
            # including across slot boundaries.
            with tc.tile_pool(name="ps_at", bufs=1, space="PSUM") as ps_at:
                tiles = [(g, k) for g in range(NSLOT) for k in range(E_OF[g])]
                ots = {}  # slot -> list of 4 psum tiles

                def emit_scores(idx):
                    g, k = tiles[idx]
                    w = _tile_width(g, k)
                    q0 = g * 256
                    st = ps_at.tile([128, 256], F32, tag="st", bufs=3, name="st")
                    for cc in range(CC):
                        nc.tensor.matmul(
                            st[:, 0:w],
                            lhsT=kt[:, cc, k * 128 : (k + 1) * 128],
                            rhs=qt[:, cc, q0 + 256 - w : q0 + 256],
                            start=(cc == 0),
                            stop=(cc == CC - 1),
                        )
                    return st

                def slot_tail(g):
                    # rowsum: combine halves, partition-reduce via ones-matmul
                    nc.vector.tensor_tensor(
                        racc_v[:], racc_v[:], racc_p[:], op=mybir.AluOpType.add
                    )
                    rs = ps_at.tile([128, 256], F32, tag="st", bufs=3, name="rs")
                    nc.tensor.matmul(
                        rs[0:1, :], lhsT=ones[:], rhs=racc_v[:], start=True, stop=True
                    )
                    rsb = work.tile([1, 256], F32, tag="rsb")
                    nc.vector.tensor_copy(rsb[:], rs[0:1, :])
                    rpp = work.tile([128, 2], F32, tag="rpp")
                    for rt in range(2):
                        nc.gpsimd.dma_start(
                            rpp[:, rt : rt + 1], rsb[0:1, rt * 128 : (rt + 1) * 128]
                        )
                    nc.vector.reciprocal(rr[:, g * 2 : g * 2 + 2], rpp[:])

                    # evacuate out^T
                    ot = ots.pop(g)
                    otsb = work.tile([128, CC, 256], BF, tag="otsb", bufs=2)
                    for cc in range(CC):
                        if cc % 2 == 0:
                            nc.scalar.copy(otsb[:, cc, :], ot[cc][:, 0:256])
                        else:
                            nc.vector.tensor_copy(otsb[:, cc, :], ot[cc][:, 0:256])
                    # projection: 2 row-tiles of 128
                    for rt in range(2):
                        yp = ps_at.tile([128, 512], F32, tag="yp", bufs=1, name="yp")
                        for cc in range(CC):
                            nc.tensor.matmul(
                                yp[:],
                                lhsT=otsb[:, cc, rt * 128 : (rt + 1) * 128],
                                rhs=wp[:, cc, :],
                                start=(cc == 0),
                                stop=(cc == CC - 1),
                            )
                        ysb = work.tile([128, 512], F32, tag="ysb")
                        nc.vector.tensor_scalar(
                            ysb[:],
                            in0=yp[:],
                            scalar1=rr[:, g * 2 + rt : g * 2 + rt + 1],
                            scalar2=None,
                            op0=mybir.AluOpType.mult,
                        )
                        r0 = g * 256 + rt * 128
                        nc.sync.dma_start(d_y.ap()[r0 : r0 + 128, :], ysb[:])

                stq = [emit_scores(0), emit_scores(1)]
                for idx, (g, k) in enumerate(tiles):
                    E = E_OF[g]
                    ms = MS_OF[g]
                    w = _tile_width(g, k)
                    s = 256 - w
                    if idx + 2 < len(tiles):
                        stq.append(emit_scores(idx + 2))
                    if k == 0:
                        # one full PSUM bank per accumulator: start=True
                        # clears has_written for the WHOLE bank, so two
                        # accumulating regions must never share a bank
                        ots[g] = [
                            ps_at.tile(
                                [128, 512], F32, tag=f"ot{cc}", name=f"ot{cc}", bufs=1
                            )
                            for cc in range(CC)
                        ]
                    st_cur = stq.pop(0)
                    e = work.tile([128, 256], BF, tag="e", bufs=4)
                    nc.scalar.activation(
                        e[:, 0:w], st_cur[:, 0:w], mybir.ActivationFunctionType.Exp
                    )
                    if k >= ms:
                        m0 = MASK_OFF[(g, k)]
                        nc.vector.tensor_tensor(
                            e[:, 0:w],
                            e[:, 0:w],
                            maskt[:, m0 : m0 + w],
                            op=mybir.AluOpType.mult,
                        )
                    # rowsum accumulate off the PE (alternate DVE / Pool)
                    racc = racc_v if k % 2 == 0 else racc_p
                    eng = nc.vector if k % 2 == 0 else nc.gpsimd
                    if k < 2:
                        eng.tensor_copy(racc[:, s:256], e[:, 0:w])
                    else:
                        eng.tensor_tensor(
                            racc[:, s:256],
                            racc[:, s:256],
                            e[:, 0:w],
                            op=mybir.AluOpType.add,
                        )
                    ot = ots[g]
                    for cc in range(CC):
                        nc.tensor.matmul(
                            ot[cc][:, s:256],
                            lhsT=vt[:, k, cc * 128 : (cc + 1) * 128],
                            rhs=e[:, 0:w],
                            start=(k == 0),
                            stop=(k == E - 1),
                        )
                    if k == E - 1:
                        slot_tail(g)

    _split_excess_waits(nc)
    return nc


_NC = None


def _get_program():
    global _NC
    if _NC is None:
        _NC = build_program()
    return _NC


LAST_RESULT = None


def _prep_xt(xrows):
    """[N,512] fp32 rows -> [128, N//512, 4, 512] bf16 (p, chunk, cc, j)"""
    n = xrows.shape[0]
    a = xrows.T.reshape(CC, 128, n)  # [cc, p, t]
    a = a.transpose(1, 0, 2)  # [p, cc, t]
    a = a.reshape(128, CC, n // 512, 512).transpose(0, 2, 1, 3)  # [p, chunk, cc, j]
    return np.ascontiguousarray(a).astype(BFNP)


def _prep_w(wT):
    """[512 in, 512 out] -> [128, 4, 512] bf16"""
    a = wT.reshape(CC, 128, C).transpose(1, 0, 2)
    return np.ascontiguousarray(a).astype(BFNP)


def _build_mask(part):
    """bf16 [128, MASK_TOT]: 1.0 where key visible for this core's rows"""
    m = np.zeros((128, MASK_TOT), dtype=np.float32)
    for g in range(NSLOT):
        base = 512 * g + 256 * part  # global row of slot-local row 0
        for k in range(MS_OF[g], E_OF[g]):
            w = _tile_width(g, k)
            off = MASK_OFF[(g, k)]
            rows = base + (256 - w) + np.arange(w)  # [w]
            keys = k * 128 + np.arange(128)  # [128]
            m[:, off : off + w] = (rows[None, :] >= keys[:, None]).astype(np.float32)
    return np.ascontiguousarray(m.astype(BFNP))


def kernel(x, w_qkv, w_proj):
    from concourse.bass_utils import run_bass_kernel_spmd

    x = np.asarray(x, dtype=np.float32)
    w_qkv = np.asarray(w_qkv, dtype=np.float32)
    w_proj = np.asarray(w_proj, dtype=np.float32)

    wqh = _prep_w(np.ascontiguousarray((w_qkv[0:C] * SCALE).T))
    wkh = _prep_w(np.ascontiguousarray(w_qkv[C : 2 * C].T))
    wvh = _prep_w(np.ascontiguousarray(w_qkv[2 * C : 3 * C].T))
    wph = _prep_w(np.ascontiguousarray(w_proj.T))
    masks = [_build_mask(0), _build_mask(1)]

    in_maps = []
    for core in range(NCORES):
        b, part = divmod(core, 2)
        xb = x[b]
        xt = _prep_xt(xb)
        # own q rows: slot g -> rows [512g + 256*part, +256)
        qrows = np.concatenate(
            [xb[512 * g + 256 * part : 512 * g + 256 * part + 256] for g in range(NSLOT)]
        )
        qxt = _prep_xt(qrows)
        in_maps.append(
            {
                "xt": xt,
                "qxt": qxt,
                "wq": wqh,
                "wk": wkh,
                "wv": wvh,
                "wp": wph,
                "mask": masks[part],
            }
        )

    global LAST_RESULT
    res = run_bass_kernel_spmd(_get_program(), in_maps, core_ids=list(range(NCORES)))
    LAST_RESULT = res

    y = np.empty((B, T, C), dtype=np.float32)
    for core in range(NCORES):
        b, part = divmod(core, 2)
        yc = res.results[core]["y"]
        for g in range(NSLOT):
            r0 = 512 * g + 256 * part
            y[b, r0 : r0 + 256, :] = yc[g * 256 : (g + 1) * 256, :]
    return y
